# revision 38
# baseline (speedup 1.0000x reference)
"""Transformer block (pre-norm attention + MLP) on 8 TRN2 NeuronCores.

Sharding: 8 cores = 4 batch elements x 2 sequence halves (data parallel, no
collectives). Each core computes its 1024 "own" query tokens end-to-end and
redundantly builds K/V for the full 2048-token batch element. The k-token
order is permuted per core (own tokens first) so the SPMD program is
identical on every core — softmax over k is permutation invariant.

All matmuls run with bf16 operands (fp32 PSUM accumulation); the residual
path stays fp32. LayerNorm affine params are folded into the adjacent matmul
weights host-side. Softmax skips max-subtraction (|scores| <= ~10 here) and
gets its denominators for free from an appended ones-column on V.
"""

from collections import deque
from contextlib import ExitStack

import numpy as np

try:
    import jax
    jax.config.update("jax_compilation_cache_dir", "/tmp/jax_bass_cache")
    jax.config.update("jax_persistent_cache_min_compile_time_secs", 0.0)
    jax.config.update("jax_persistent_cache_min_entry_size_bytes", -1)
except Exception:
    import jax

import concourse.bacc as bacc
import concourse.bass as bass
import concourse.mybir as mybir
import concourse.tile as tile
from concourse.masks import make_identity

FP32 = mybir.dt.float32
BF16 = mybir.dt.bfloat16
AF = mybir.ActivationFunctionType
ALU = mybir.AluOpType

D = 1024          # model dim
DT = 8            # d tiles of 128
H = 16            # heads
HD = 64           # head dim
HID = 4096        # mlp hidden
T_ALL = 2048      # tokens per core incl. K/V-only tokens
T_OWN = 1024      # query/output tokens per core
EPS = 1e-6
N_CORES = 8


def _ln_transpose(nc, statp, znp, trp, src_getter, n_tiles, zt_out, eps_sb, ident,
                  copies_on_act=False, apply_on_gpsimd=False):
    """LayerNorm (w/b folded into the following matmul weights host-side)
    + PE transpose into zt_out [128, DT, n_tiles*128] bf16."""
    for tt in range(n_tiles):
        xt = src_getter(tt)  # [128, D] fp32 sbuf tile
        stats = statp.tile([128, 2, 6], FP32, tag="stats")
        nc.vector.bn_stats(out=stats[:, 0, :], in_=xt[:, 0:512])
        nc.vector.bn_stats(out=stats[:, 1, :], in_=xt[:, 512:1024])
        mv = statp.tile([128, 2], FP32, tag="mv")
        nc.vector.bn_aggr(out=mv, in_=stats)
        sd = statp.tile([128, 1], FP32, tag="sd")
        nc.scalar.activation(out=sd, in_=mv[:, 1:2], func=AF.Sqrt, bias=eps_sb)
        rinv = statp.tile([128, 1], FP32, tag="rinv")
        nc.vector.reciprocal(out=rinv, in_=sd)
        zn = znp.tile([128, D], BF16, tag="zn")
        eng = nc.gpsimd if apply_on_gpsimd else nc.vector
        eng.tensor_scalar(
            out=zn, in0=xt, scalar1=mv[:, 0:1], scalar2=rinv,
            op0=ALU.subtract, op1=ALU.mult,
        )
        if trp is None:
            # bf16 xbar DMA transpose: no PE work, no PSUM bounce
            for d in range(DT):
                nc.sync.dma_start_transpose(
                    out=zt_out[:, d, tt * 128:(tt + 1) * 128],
                    in_=zn[:, d * 128:(d + 1) * 128],
                )
        else:
            for g in range(2):
                ps = trp.tile([128, 4, 128], BF16, tag="trps")
                for i in range(4):
                    nc.tensor.transpose(
                        ps[:, i, :],
                        zn[:, (4 * g + i) * 128:(4 * g + i + 1) * 128], ident
                    )
                cp = nc.scalar.copy if copies_on_act else nc.vector.tensor_copy
                cp(out=zt_out[:, 4 * g:4 * g + 4, tt * 128:(tt + 1) * 128],
                   in_=ps)


def build_nc():
    nc = bacc.Bacc("TRN2", target_bir_lowering=False, debug=False, num_devices=N_CORES)

    x = nc.dram_tensor("x", [T_ALL, D], FP32, kind="ExternalInput")
    wqkv = nc.dram_tensor("wqkv", [D, 3 * D], BF16, kind="ExternalInput")
    bqkv = nc.dram_tensor("bqkv", [3 * D], FP32, kind="ExternalInput")
    wproj = nc.dram_tensor("wproj", [D, D], BF16, kind="ExternalInput")
    bproj = nc.dram_tensor("bproj", [D], BF16, kind="ExternalInput")
    w1 = nc.dram_tensor("w1", [D, HID], BF16, kind="ExternalInput")
    b1 = nc.dram_tensor("b1", [HID], FP32, kind="ExternalInput")
    w2 = nc.dram_tensor("w2", [HID, D], BF16, kind="ExternalInput")
    b2 = nc.dram_tensor("b2", [D], BF16, kind="ExternalInput")
    y = nc.dram_tensor("y", [T_OWN, D], FP32, kind="ExternalOutput")
    wqkv_t = wqkv.ap().rearrange("(dt p) f -> p dt f", p=128)   # [128, 8, 3072]
    w1_t = w1.ap().rearrange("(dt p) f -> p dt f", p=128)       # [128, 8, 4096]
    w2_t = w2.ap().rearrange("(jt p) f -> p jt f", p=128)       # [128, 32, 1024]

    with tile.TileContext(nc) as tc, ExitStack() as ctx:
        P = ctx.enter_context

        # ---- whole-kernel pools ----
        singles = P(tc.tile_pool(name="singles", bufs=1))
        xpool = P(tc.tile_pool(name="xin", bufs=3))
        statpool = P(tc.tile_pool(name="stat", bufs=6))
        znpool = P(tc.tile_pool(name="zn", bufs=4))
        es_ao = ExitStack()
        aop = es_ao.enter_context(tc.tile_pool(name="aop", bufs=1, side="right"))
        es_pjw = ExitStack()
        pjw_pool = es_pjw.enter_context(
            tc.tile_pool(name="pjw", bufs=1, side="right"))

        # ---- constants ----
        ident = singles.tile([128, 128], BF16)
        make_identity(nc, ident)
        ones_bf = singles.tile([1, 128], BF16)
        nc.vector.memset(ones_bf, 1.0)
        eps_sb = singles.tile([128, 1], FP32)
        nc.vector.memset(eps_sb, EPS)
        bq_sb = singles.tile([128, 24], FP32)
        nc.sync.dma_start(out=bq_sb, in_=bqkv.ap().rearrange("(f p) -> p f", p=128))
        b1_sb = singles.tile([128, 32], FP32)
        nc.sync.dma_start(out=b1_sb, in_=b1.ap().rearrange("(f p) -> p f", p=128))
        bproj_sb = singles.tile([1, D], BF16)
        nc.sync.dma_start(out=bproj_sb, in_=bproj.ap().rearrange("(o f) -> o f", o=1))
        b2_sb = singles.tile([1, D], BF16)
        nc.sync.dma_start(out=b2_sb, in_=b2.ap().rearrange("(o f) -> o f", o=1))
        # V-bias broadcast to all partitions [128, 1024]
        vbias_sb = singles.tile([128, D], FP32)
        nc.sync.dma_start(
            out=vbias_sb,
            in_=bass.AP(tensor=bqkv, offset=2 * D, ap=[[0, 128], [1, D]]),
        )

        # ---- phase A: LN1 + transpose -> z1T ----
        es_z1 = ExitStack()
        z1p = es_z1.enter_context(tc.tile_pool(name="z1p", bufs=1, side="right"))
        z1T = z1p.tile([128, DT, T_ALL], BF16, tag="z1T")

        def load_x(tt):
            xt = xpool.tile([128, D], FP32, tag="xa")
            nc.sync.dma_start(out=xt, in_=x[tt * 128:(tt + 1) * 128, :])
            return xt

        with tc.tile_pool(name="psA", bufs=2, space="PSUM") as trpsA:
            _ln_transpose(nc, statpool, znpool, trpsA, load_x, T_ALL // 128,
                          z1T, eps_sb, ident, copies_on_act=True)

        # proj weights: prefetch now (used ~300us later in phase D)
        projw_sb = pjw_pool.tile([128, DT, D], BF16, tag="projw")
        nc.sync.dma_start(
            out=projw_sb, in_=wproj.ap().rearrange("(dt p) f -> p dt f", p=128)
        )

        # ---- fused QKV + attention ----
        es_kqv = ExitStack()
        kqvp = es_kqv.enter_context(tc.tile_pool(name="kqvp", bufs=1))
        kt_all = kqvp.tile([128, DT, T_ALL], BF16, tag="kt")
        qt_all = kqvp.tile([128, DT, T_OWN], BF16, tag="qt")
        VP = kqvp.tile([128, 16, 16 * (HD + 1)], BF16, tag="vp")
        vp_ones = VP.rearrange("p k (h e) -> p k h e", e=HD + 1)[:, :, :, HD:HD + 1]
        nc.vector.memset(vp_ones, 1.0)
        aoT = aop.tile([128, DT, T_OWN], BF16, tag="aoT")

        with (
            tc.tile_pool(name="wq", bufs=2, side="right") as wq_pool,
            tc.tile_pool(name="wv", bufs=1, side="right") as wv_pool,
            tc.tile_pool(name="psB", bufs=2, space="PSUM") as qkpsum,
            tc.tile_pool(name="exps", bufs=4) as exp_pool,
            tc.tile_pool(name="nrm", bufs=2) as nrm_pool,
            tc.tile_pool(name="psCs", bufs=2, space="PSUM") as spsum,
            tc.tile_pool(name="psCa", bufs=2, space="PSUM") as avpsum,
            tc.tile_pool(name="drp", bufs=3, space="DRAM") as drpool,
        ):
            def kq_fillers(j):
                out = []
                state = {}
                for f in (8 + j, j):
                    nch = 4 if f >= 8 else 2
                    for tcn in range(nch):
                        def grp(f=f, tcn=tcn):
                            if f not in state:
                                wq_f = wq_pool.tile([128, DT, 128], BF16,
                                                    tag="wqf")
                                nc.sync.dma_start(
                                    out=wq_f,
                                    in_=wqkv_t[:, :, f * 128:(f + 1) * 128])
                                state[f] = wq_f
                            wq_f = state[f]
                            ps = qkpsum.tile([128, 512], FP32, tag="qkps")
                            for d in range(DT):
                                nc.tensor.matmul(
                                    ps, wq_f[:, d, :],
                                    z1T[:, d, tcn * 512:(tcn + 1) * 512],
                                    start=(d == 0), stop=(d == DT - 1),
                                )
                            if f >= 8:
                                dst = kt_all[:, f - 8, tcn * 512:(tcn + 1) * 512]
                            else:
                                dst = qt_all[:, f, tcn * 512:(tcn + 1) * 512]
                            nc.vector.tensor_scalar(
                                out=dst, in0=ps, scalar1=bq_sb[:, f:f + 1],
                                scalar2=None, op0=ALU.add,
                            )
                        out.append(grp)
                return out

            def v_fillers(vc):
                out = []
                state = {}
                for tt in range(T_ALL // 128):
                    def grp(tt=tt):
                        if "wv" not in state:
                            wv = wv_pool.tile([128, DT, 512], BF16, tag="wvf")
                            nc.sync.dma_start(
                                out=wv,
                                in_=wqkv_t[:, :, 2 * D + vc * 512:
                                           2 * D + (vc + 1) * 512])
                            state["wv"] = wv
                        wv = state["wv"]
                        ps = qkpsum.tile([128, 512], FP32, tag="qkps")
                        for d in range(DT):
                            nc.tensor.matmul(
                                ps, z1T[:, d, tt * 128:(tt + 1) * 128],
                                wv[:, d, :],
                                start=(d == 0), stop=(d == DT - 1),
                            )
                        dst = VP[:, tt, vc * 8 * (HD + 1):
                                 (vc + 1) * 8 * (HD + 1)]
                        dst = dst.rearrange(
                            "p (h e) -> p h e", e=HD + 1)[:, :, 0:HD]
                        srcp = ps.rearrange("p (h e) -> p h e", e=HD)
                        vb = vbias_sb[:, vc * 512:(vc + 1) * 512].rearrange(
                            "p (h e) -> p h e", e=HD)
                        nc.vector.scalar_tensor_tensor(
                            out=dst, in0=srcp, scalar=0.0, in1=vb,
                            op0=ALU.bypass, op1=ALU.add,
                        )
                    out.append(grp)
                return out

            def emit_pair_fill(j, fillers, per_kt=None):
                """Process head pair (2j, 2j+1) with scores row-packed into
                the two 64-row halves of the PE array (tile_position), one
                q-chunk at a time so PSUM stays within 8 banks. Filler
                psum-groups are popped every 3rd kt step."""
                h0, h1 = 2 * j, 2 * j + 1
                for qc in range(2):
                    avs = []
                    for hh in (h0, h1):
                        av_t = avpsum.tile([HD + 1, 512], FP32, tag="av")
                        avs.append(av_t)
                    for kt in range(T_ALL // 128):
                        sp = spsum.tile([128, T_OWN], FP32, tag="sps")
                        for hi, hh in enumerate((h0, h1)):
                            pr = hi * 64
                            nc.tensor.matmul(
                                sp[:, hi * 512:(hi + 1) * 512],
                                kt_all[pr:pr + 64, j, kt * 128:(kt + 1) * 128],
                                qt_all[pr:pr + 64, j, qc * 512:(qc + 1) * 512],
                                start=True, stop=True,
                                tile_position=(pr, 0),
                            )
                        ex = exp_pool.tile([128, T_OWN], BF16, tag="exp")
                        nc.scalar.activation(out=ex, in_=sp, func=AF.Exp,
                                             scale=0.125)
                        if per_kt is not None and qc == 0:
                            per_kt(kt)
                        for hi, hh in enumerate((h0, h1)):
                            nc.tensor.matmul(
                                avs[hi],
                                VP[:, kt, hh * (HD + 1):(hh + 1) * (HD + 1)],
                                ex[:, hi * 512:(hi + 1) * 512],
                                start=(kt == 0), stop=(kt == T_ALL // 128 - 1),
                            )
                        if kt % 3 == 2 and fillers:
                            fillers.popleft()()
                    for hi, hh in enumerate((h0, h1)):
                        av = avs[hi]
                        ft, pr = hh // 2, (hh % 2) * 64
                        asl = aoT[pr:pr + 64, ft, qc * 512:(qc + 1) * 512]
                        nc.vector.tensor_copy(out=asl, in_=av[0:HD, :])
                        sums_sb = nrm_pool.tile([1, 512], FP32, tag="sums")
                        nc.vector.tensor_copy(out=sums_sb, in_=av[HD:HD + 1, :])
                        rec = nrm_pool.tile([1, 512], FP32, tag="rec")
                        nc.vector.reciprocal_approx_fast(out=rec, in_=sums_sb)
                        rec_bf = nrm_pool.tile([1, 512], BF16, tag="recbf")
                        nc.vector.tensor_copy(out=rec_bf, in_=rec)
                        drt = drpool.tile([1, 512], BF16, tag="drrec")
                        nc.sync.dma_start(out=drt, in_=rec_bf)
                        bcs = nrm_pool.tile([128, 512], BF16, tag="bcs")
                        nc.sync.dma_start(out=bcs,
                                          in_=drt.broadcast_to([128, 512]))
                        nc.vector.tensor_mul(
                            out=asl, in0=asl, in1=bcs[pr:pr + HD, :]
                        )

            vf1 = v_fillers(1)
            for f in kq_fillers(0):
                f()
            v0 = v_fillers(0)
            for j in range(8):
                fillers = deque()
                if j + 1 < 8:
                    fillers.extend(kq_fillers(j + 1))
                if j < 4:
                    fillers.extend(vf1[j * 4:(j + 1) * 4])
                if j == 0:
                    # pair 0 drives V(vc0, kt) just-in-time for its attnV
                    emit_pair_fill(0, fillers, per_kt=lambda kt: v0[kt]())
                else:
                    emit_pair_fill(j, fillers)
                # flush leftover fillers before next pair needs them
                while fillers:
                    fillers.popleft()()
        es_z1.close()  # z1T dead
        es_kqv.close()  # kt/qt/VP dead

        # ---- phase D: proj + residual -> x2 ----
        es_x2 = ExitStack()
        x2p = es_x2.enter_context(tc.tile_pool(name="x2p", bufs=1))
        x2_all = x2p.tile([128, T_OWN // 128, D], FP32, tag="x2")
        with tc.tile_pool(name="psD", bufs=2, space="PSUM") as ppsum:
            for tt in range(T_OWN // 128):
                xo = xpool.tile([128, D], FP32, tag="xa")
                nc.sync.dma_start(out=xo, in_=x[tt * 128:(tt + 1) * 128, :])
                for oc in range(2):
                    ps = ppsum.tile([128, 512], FP32, tag="pps")
                    for d in range(DT):
                        nc.tensor.matmul(
                            ps, aoT[:, d, tt * 128:(tt + 1) * 128],
                            projw_sb[:, d, oc * 512:(oc + 1) * 512],
                            start=(d == 0), stop=False,
                        )
                    nc.tensor.matmul(
                        ps, ones_bf, bproj_sb[:, oc * 512:(oc + 1) * 512],
                        start=False, stop=True,
                    )
                    nc.vector.scalar_tensor_tensor(
                        out=x2_all[:, tt, oc * 512:(oc + 1) * 512],
                        in0=ps, scalar=0.0, in1=xo[:, oc * 512:(oc + 1) * 512],
                        op0=ALU.bypass, op1=ALU.add,
                    )
        es_pjw.close()
        es_ao.close()  # aoT dead

        # ---- phase E: LN2 + transpose -> z2T ----
        es_z2 = ExitStack()
        z2p = es_z2.enter_context(tc.tile_pool(name="z2p", bufs=1))
        z2T = z2p.tile([128, DT, T_OWN], BF16, tag="z2T")
        with tc.tile_pool(name="psE", bufs=2, space="PSUM") as trpsE:
            _ln_transpose(nc, statpool, znpool, trpsE,
                          lambda tt: x2_all[:, tt, :], T_OWN // 128,
                          z2T, eps_sb, ident)

        # ---- phase F: MLP ----
        with (
            tc.tile_pool(name="yp", bufs=3) as ypool,
            tc.tile_pool(name="w1p", bufs=3) as w1_pool,
            tc.tile_pool(name="w2p", bufs=1) as w2_pool,
            tc.tile_pool(name="hp", bufs=1) as hpool,
            tc.tile_pool(name="psF", bufs=6, space="PSUM") as fpsum,
        ):
            w2_sb = w2_pool.tile([128, HID // 128, D], BF16, tag="w2sb")
            nc.sync.dma_start(out=w2_sb, in_=w2_t)
            for tc2 in range(2):
                hT = hpool.tile([128, HID // 128, 512], BF16, tag="hT")
                for jt in range(HID // 128):
                    w1f = w1_pool.tile([128, DT, 128], BF16, tag="w1f")
                    nc.sync.dma_start(
                        out=w1f, in_=w1_t[:, :, jt * 128:(jt + 1) * 128]
                    )
                    ps = fpsum.tile([128, 512], FP32, tag="fps")
                    for d in range(DT):
                        nc.tensor.matmul(
                            ps, w1f[:, d, :], z2T[:, d, tc2 * 512:(tc2 + 1) * 512],
                            start=(d == 0), stop=(d == DT - 1),
                        )
                    nc.scalar.activation(
                        out=hT[:, jt, :], in_=ps, func=AF.Gelu,
                        bias=b1_sb[:, jt:jt + 1],
                    )
                for oc in range(2):
                    pss = []
                    for i in range(4):
                        ops_t = fpsum.tile([128, 512], FP32, tag="fps")
                        pss.append(ops_t)
                    for jt in range(HID // 128):
                        for tt in range(4):
                            nc.tensor.matmul(
                                pss[tt], hT[:, jt, tt * 128:(tt + 1) * 128],
                                w2_sb[:, jt, oc * 512:(oc + 1) * 512],
                                start=(jt == 0), stop=False,
                            )
                    for tt in range(4):
                        tglob = tc2 * 4 + tt
                        nc.tensor.matmul(
                            pss[tt], ones_bf, b2_sb[:, oc * 512:(oc + 1) * 512],
                            start=False, stop=True,
                        )
                        ys = ypool.tile([128, 512], FP32, tag="ys")
                        nc.vector.scalar_tensor_tensor(
                            out=ys, in0=pss[tt], scalar=0.0,
                            in1=x2_all[:, tglob, oc * 512:(oc + 1) * 512],
                            op0=ALU.bypass, op1=ALU.add,
                        )
                        nc.sync.dma_start(
                            out=y[tglob * 128:(tglob + 1) * 128,
                                  oc * 512:(oc + 1) * 512],
                            in_=ys,
                        )
        es_z2.close()
        es_x2.close()

    nc.compile()
    return nc


def prep_host_inputs(inputs):
    """Fold LN affine params into the adjacent matmul weights, cast to bf16,
    and build the 8 per-core input maps."""
    import ml_dtypes

    f32 = np.float32
    x = np.asarray(inputs["x"], f32)
    qkv_w = np.asarray(inputs["qkv_w"], f32)
    qkv_b = np.asarray(inputs["qkv_b"], f32)
    proj_w = np.asarray(inputs["proj_w"], f32)
    proj_b = np.asarray(inputs["proj_b"], f32)
    fc1_w = np.asarray(inputs["fc1_w"], f32)
    fc1_b = np.asarray(inputs["fc1_b"], f32)
    fc2_w = np.asarray(inputs["fc2_w"], f32)
    fc2_b = np.asarray(inputs["fc2_b"], f32)
    ln1_w = np.asarray(inputs["ln1_w"], f32)
    ln1_b = np.asarray(inputs["ln1_b"], f32)
    ln2_w = np.asarray(inputs["ln2_w"], f32)
    ln2_b = np.asarray(inputs["ln2_b"], f32)

    bf = ml_dtypes.bfloat16
    wqkv = (ln1_w[:, None] * qkv_w).astype(bf)
    bqkv = (qkv_b + ln1_b @ qkv_w).astype(f32)
    w1 = (ln2_w[:, None] * fc1_w).astype(bf)
    b1 = (fc1_b + ln2_b @ fc1_w).astype(f32)

    shared = {
        "wqkv": wqkv, "bqkv": bqkv,
        "wproj": proj_w.astype(bf), "bproj": proj_b.astype(bf),
        "w1": w1, "b1": b1,
        "w2": fc2_w.astype(bf), "b2": fc2_b.astype(bf),
    }
    in_maps = []
    for c in range(N_CORES):
        b, half = c // 2, c % 2
        own = x[b, half * 1024:(half + 1) * 1024]
        other = x[b, (1 - half) * 1024:(2 - half) * 1024]
        xc = np.concatenate([own, other], axis=0)
        in_maps.append({"x": np.ascontiguousarray(xc), **shared})
    return in_maps


# ---------------------------------------------------------------------------
# Cached PJRT runner (jit once, reuse across kernel() calls)
# ---------------------------------------------------------------------------
_CACHE = {}


def _get_runner():
    if "runner" in _CACHE:
        return _CACHE["runner"]

    from jax.experimental.shard_map import shard_map
    from jax.sharding import Mesh, PartitionSpec
    from concourse.bass2jax import (
        _bass_exec_p, install_neuronx_cc_hook, partition_id_tensor,
    )

    nc = build_nc()
    install_neuronx_cc_hook()

    partition_name = nc.partition_id_tensor.name if nc.partition_id_tensor else None
    in_names, out_names, out_avals, zero_shapes = [], [], [], []
    for alloc in nc.m.functions[0].allocations:
        if not isinstance(alloc, mybir.MemoryLocationSet):
            continue
        name = alloc.memorylocations[0].name
        if alloc.kind == "ExternalInput":
            if name != partition_name:
                in_names.append(name)
        elif alloc.kind == "ExternalOutput":
            shape = tuple(alloc.tensor_shape)
            dtype = mybir.dt.np(alloc.dtype)
            out_names.append(name)
            out_avals.append(jax.core.ShapedArray(shape, dtype))
            zero_shapes.append((shape, dtype))
    n_params = len(in_names)
    n_outs = len(out_names)
    all_in = list(in_names) + list(out_names)
    if partition_name is not None:
        all_in.append(partition_name)
    donate = tuple(range(n_params, n_params + n_outs))

    def _body(*args):
        operands = list(args)
        if partition_name is not None:
            operands.append(partition_id_tensor())
        outs = _bass_exec_p.bind(
            *operands,
            out_avals=tuple(out_avals),
            in_names=tuple(all_in),
            out_names=tuple(out_names),
            lowering_input_output_aliases=(),
            sim_require_finite=True,
            sim_require_nnan=True,
            nc=nc,
        )
        return tuple(outs)

    devices = jax.devices()[:N_CORES]
    mesh = Mesh(np.asarray(devices), ("core",))
    sharded = jax.jit(
        shard_map(
            _body, mesh=mesh,
            in_specs=(PartitionSpec("core"),) * (n_params + n_outs),
            out_specs=(PartitionSpec("core"),) * n_outs,
            check_rep=False,
        ),
        donate_argnums=donate, keep_unused=True,
    )

    def run(in_maps):
        concat_in = [
            np.concatenate([np.asarray(m[name]) for m in in_maps], axis=0)
            for name in in_names
        ]
        concat_zeros = [
            np.zeros((N_CORES * s[0], *s[1:]), dt) for (s, dt) in zero_shapes
        ]
        out_arrs = sharded(*concat_in, *concat_zeros)
        per_core = []
        for c in range(N_CORES):
            per_core.append({
                name: np.asarray(out_arrs[i]).reshape(
                    N_CORES, *out_avals[i].shape)[c]
                for i, name in enumerate(out_names)
            })
        return per_core

    _CACHE["runner"] = run
    return run


def kernel(**inputs) -> np.ndarray:
    run = _get_runner()
    in_maps = prep_host_inputs(inputs)
    results = run(in_maps)
    out = np.zeros((4, 2048, 1024), np.float32)
    for c in range(N_CORES):
        b, half = c // 2, c % 2
        out[b, half * 1024:(half + 1) * 1024] = results[c]["y"]
    return out



# revision 39
# speedup vs baseline: 1.0132x; 1.0132x over previous
"""Transformer block (pre-norm attention + MLP) on 8 TRN2 NeuronCores.

Sharding: 8 cores = 4 batch elements x 2 sequence halves (data parallel, no
collectives). Each core computes its 1024 "own" query tokens end-to-end and
redundantly builds K/V for the full 2048-token batch element. The k-token
order is permuted per core (own tokens first) so the SPMD program is
identical on every core — softmax over k is permutation invariant.

All matmuls run with bf16 operands (fp32 PSUM accumulation); the residual
path stays fp32. LayerNorm affine params are folded into the adjacent matmul
weights host-side. Softmax skips max-subtraction (|scores| <= ~10 here) and
gets its denominators for free from an appended ones-column on V.
"""

from collections import deque
from contextlib import ExitStack

import numpy as np

try:
    import jax
    jax.config.update("jax_compilation_cache_dir", "/tmp/jax_bass_cache")
    jax.config.update("jax_persistent_cache_min_compile_time_secs", 0.0)
    jax.config.update("jax_persistent_cache_min_entry_size_bytes", -1)
except Exception:
    import jax

import concourse.bacc as bacc
import concourse.bass as bass
import concourse.mybir as mybir
import concourse.tile as tile
from concourse.masks import make_identity

FP32 = mybir.dt.float32
BF16 = mybir.dt.bfloat16
AF = mybir.ActivationFunctionType
ALU = mybir.AluOpType

D = 1024          # model dim
DT = 8            # d tiles of 128
H = 16            # heads
HD = 64           # head dim
HID = 4096        # mlp hidden
T_ALL = 2048      # tokens per core incl. K/V-only tokens
T_OWN = 1024      # query/output tokens per core
EPS = 1e-6
N_CORES = 8


def _ln_transpose(nc, statp, znp, trp, src_getter, n_tiles, zt_out, eps_sb, ident,
                  copies_on_act=False, apply_on_gpsimd=False):
    """LayerNorm (w/b folded into the following matmul weights host-side)
    + PE transpose into zt_out [128, DT, n_tiles*128] bf16."""
    for tt in range(n_tiles):
        xt = src_getter(tt)  # [128, D] fp32 sbuf tile
        stats = statp.tile([128, 2, 6], FP32, tag="stats")
        nc.vector.bn_stats(out=stats[:, 0, :], in_=xt[:, 0:512])
        nc.vector.bn_stats(out=stats[:, 1, :], in_=xt[:, 512:1024])
        mv = statp.tile([128, 2], FP32, tag="mv")
        nc.vector.bn_aggr(out=mv, in_=stats)
        sd = statp.tile([128, 1], FP32, tag="sd")
        nc.scalar.activation(out=sd, in_=mv[:, 1:2], func=AF.Sqrt, bias=eps_sb)
        rinv = statp.tile([128, 1], FP32, tag="rinv")
        nc.vector.reciprocal(out=rinv, in_=sd)
        zn = znp.tile([128, D], BF16, tag="zn")
        eng = nc.gpsimd if apply_on_gpsimd else nc.vector
        eng.tensor_scalar(
            out=zn, in0=xt, scalar1=mv[:, 0:1], scalar2=rinv,
            op0=ALU.subtract, op1=ALU.mult,
        )
        if trp is None:
            # bf16 xbar DMA transpose: no PE work, no PSUM bounce
            for d in range(DT):
                nc.sync.dma_start_transpose(
                    out=zt_out[:, d, tt * 128:(tt + 1) * 128],
                    in_=zn[:, d * 128:(d + 1) * 128],
                )
        else:
            for g in range(2):
                ps = trp.tile([128, 4, 128], BF16, tag="trps")
                for i in range(4):
                    nc.tensor.transpose(
                        ps[:, i, :],
                        zn[:, (4 * g + i) * 128:(4 * g + i + 1) * 128], ident
                    )
                cp = nc.scalar.copy if copies_on_act else nc.vector.tensor_copy
                cp(out=zt_out[:, 4 * g:4 * g + 4, tt * 128:(tt + 1) * 128],
                   in_=ps)


def build_nc():
    nc = bacc.Bacc("TRN2", target_bir_lowering=False, debug=False, num_devices=N_CORES)

    x = nc.dram_tensor("x", [T_ALL, D], FP32, kind="ExternalInput")
    wqkv = nc.dram_tensor("wqkv", [D, 3 * D], BF16, kind="ExternalInput")
    bqkv = nc.dram_tensor("bqkv", [3 * D], FP32, kind="ExternalInput")
    wproj = nc.dram_tensor("wproj", [D, D], BF16, kind="ExternalInput")
    bproj = nc.dram_tensor("bproj", [D], BF16, kind="ExternalInput")
    w1a = nc.dram_tensor("w1a", [128, HID // 128, DT, 128], mybir.dt.float8e4, kind="ExternalInput")
    w1b = nc.dram_tensor("w1b", [128, HID // 128, DT, 128], mybir.dt.float8e4, kind="ExternalInput")
    b1 = nc.dram_tensor("b1", [HID], FP32, kind="ExternalInput")
    w2 = nc.dram_tensor("w2", [HID, D], BF16, kind="ExternalInput")
    b2 = nc.dram_tensor("b2", [D], BF16, kind="ExternalInput")
    y = nc.dram_tensor("y", [T_OWN, D], FP32, kind="ExternalOutput")
    wqkv_t = wqkv.ap().rearrange("(dt p) f -> p dt f", p=128)   # [128, 8, 3072]
    w2_t = w2.ap().rearrange("(jt p) f -> p jt f", p=128)       # [128, 32, 1024]

    with tile.TileContext(nc) as tc, ExitStack() as ctx:
        P = ctx.enter_context

        # ---- whole-kernel pools ----
        singles = P(tc.tile_pool(name="singles", bufs=1))
        xpool = P(tc.tile_pool(name="xin", bufs=3))
        statpool = P(tc.tile_pool(name="stat", bufs=6))
        znpool = P(tc.tile_pool(name="zn", bufs=4))
        es_ao = ExitStack()
        aop = es_ao.enter_context(tc.tile_pool(name="aop", bufs=1, side="right"))
        es_pjw = ExitStack()
        pjw_pool = es_pjw.enter_context(
            tc.tile_pool(name="pjw", bufs=1, side="right"))

        # ---- constants ----
        ident = singles.tile([128, 128], BF16)
        make_identity(nc, ident)
        ones_bf = singles.tile([1, 128], BF16)
        nc.vector.memset(ones_bf, 1.0)
        eps_sb = singles.tile([128, 1], FP32)
        nc.vector.memset(eps_sb, EPS)
        bq_sb = singles.tile([128, 24], FP32)
        nc.sync.dma_start(out=bq_sb, in_=bqkv.ap().rearrange("(f p) -> p f", p=128))
        b1_sb = singles.tile([128, 32], FP32)
        nc.sync.dma_start(out=b1_sb, in_=b1.ap().rearrange("(f p) -> p f", p=128))
        bproj_sb = singles.tile([1, D], BF16)
        nc.sync.dma_start(out=bproj_sb, in_=bproj.ap().rearrange("(o f) -> o f", o=1))
        b2_sb = singles.tile([1, D], BF16)
        nc.sync.dma_start(out=b2_sb, in_=b2.ap().rearrange("(o f) -> o f", o=1))
        # V-bias broadcast to all partitions [128, 1024]
        vbias_sb = singles.tile([128, D], FP32)
        nc.sync.dma_start(
            out=vbias_sb,
            in_=bass.AP(tensor=bqkv, offset=2 * D, ap=[[0, 128], [1, D]]),
        )

        # ---- phase A: LN1 + transpose -> z1T ----
        es_z1 = ExitStack()
        z1p = es_z1.enter_context(tc.tile_pool(name="z1p", bufs=1, side="right"))
        z1T = z1p.tile([128, DT, T_ALL], BF16, tag="z1T")

        def load_x(tt):
            xt = xpool.tile([128, D], FP32, tag="xa")
            nc.sync.dma_start(out=xt, in_=x[tt * 128:(tt + 1) * 128, :])
            return xt

        with tc.tile_pool(name="psA", bufs=2, space="PSUM") as trpsA:
            _ln_transpose(nc, statpool, znpool, trpsA, load_x, T_ALL // 128,
                          z1T, eps_sb, ident, copies_on_act=True)

        # proj weights: prefetch now (used ~300us later in phase D)
        projw_sb = pjw_pool.tile([128, DT, D], BF16, tag="projw")
        nc.sync.dma_start(
            out=projw_sb, in_=wproj.ap().rearrange("(dt p) f -> p dt f", p=128)
        )

        # ---- fused QKV + attention ----
        es_kqv = ExitStack()
        kqvp = es_kqv.enter_context(tc.tile_pool(name="kqvp", bufs=1))
        kt_all = kqvp.tile([128, DT, T_ALL], BF16, tag="kt")
        qt_all = kqvp.tile([128, DT, T_OWN], BF16, tag="qt")
        VP = kqvp.tile([128, 16, 16 * (HD + 1)], BF16, tag="vp")
        vp_ones = VP.rearrange("p k (h e) -> p k h e", e=HD + 1)[:, :, :, HD:HD + 1]
        nc.vector.memset(vp_ones, 1.0)
        aoT = aop.tile([128, DT, T_OWN], BF16, tag="aoT")

        with (
            tc.tile_pool(name="wq", bufs=2, side="right") as wq_pool,
            tc.tile_pool(name="wv", bufs=1, side="right") as wv_pool,
            tc.tile_pool(name="psB", bufs=2, space="PSUM") as qkpsum,
            tc.tile_pool(name="exps", bufs=4) as exp_pool,
            tc.tile_pool(name="nrm", bufs=2) as nrm_pool,
            tc.tile_pool(name="psCs", bufs=2, space="PSUM") as spsum,
            tc.tile_pool(name="psCa", bufs=2, space="PSUM") as avpsum,
            tc.tile_pool(name="drp", bufs=3, space="DRAM") as drpool,
        ):
            def kq_fillers(j):
                out = []
                state = {}
                for f in (8 + j, j):
                    nch = 4 if f >= 8 else 2
                    for tcn in range(nch):
                        def grp(f=f, tcn=tcn):
                            if f not in state:
                                wq_f = wq_pool.tile([128, DT, 128], BF16,
                                                    tag="wqf")
                                nc.sync.dma_start(
                                    out=wq_f,
                                    in_=wqkv_t[:, :, f * 128:(f + 1) * 128])
                                state[f] = wq_f
                            wq_f = state[f]
                            ps = qkpsum.tile([128, 512], FP32, tag="qkps")
                            for d in range(DT):
                                nc.tensor.matmul(
                                    ps, wq_f[:, d, :],
                                    z1T[:, d, tcn * 512:(tcn + 1) * 512],
                                    start=(d == 0), stop=(d == DT - 1),
                                )
                            if f >= 8:
                                dst = kt_all[:, f - 8, tcn * 512:(tcn + 1) * 512]
                            else:
                                dst = qt_all[:, f, tcn * 512:(tcn + 1) * 512]
                            nc.vector.tensor_scalar(
                                out=dst, in0=ps, scalar1=bq_sb[:, f:f + 1],
                                scalar2=None, op0=ALU.add,
                            )
                        out.append(grp)
                return out

            def v_fillers(vc):
                out = []
                state = {}
                for tt in range(T_ALL // 128):
                    def grp(tt=tt):
                        if "wv" not in state:
                            wv = wv_pool.tile([128, DT, 512], BF16, tag="wvf")
                            nc.sync.dma_start(
                                out=wv,
                                in_=wqkv_t[:, :, 2 * D + vc * 512:
                                           2 * D + (vc + 1) * 512])
                            state["wv"] = wv
                        wv = state["wv"]
                        ps = qkpsum.tile([128, 512], FP32, tag="qkps")
                        for d in range(DT):
                            nc.tensor.matmul(
                                ps, z1T[:, d, tt * 128:(tt + 1) * 128],
                                wv[:, d, :],
                                start=(d == 0), stop=(d == DT - 1),
                            )
                        dst = VP[:, tt, vc * 8 * (HD + 1):
                                 (vc + 1) * 8 * (HD + 1)]
                        dst = dst.rearrange(
                            "p (h e) -> p h e", e=HD + 1)[:, :, 0:HD]
                        srcp = ps.rearrange("p (h e) -> p h e", e=HD)
                        vb = vbias_sb[:, vc * 512:(vc + 1) * 512].rearrange(
                            "p (h e) -> p h e", e=HD)
                        nc.vector.scalar_tensor_tensor(
                            out=dst, in0=srcp, scalar=0.0, in1=vb,
                            op0=ALU.bypass, op1=ALU.add,
                        )
                    out.append(grp)
                return out

            def emit_pair_fill(j, fillers, per_kt=None):
                """Process head pair (2j, 2j+1) with scores row-packed into
                the two 64-row halves of the PE array (tile_position), one
                q-chunk at a time so PSUM stays within 8 banks. Filler
                psum-groups are popped every 3rd kt step."""
                h0, h1 = 2 * j, 2 * j + 1
                for qc in range(2):
                    avs = []
                    for hh in (h0, h1):
                        av_t = avpsum.tile([HD + 1, 512], FP32, tag="av")
                        avs.append(av_t)
                    for kt in range(T_ALL // 128):
                        sp = spsum.tile([128, T_OWN], FP32, tag="sps")
                        for hi, hh in enumerate((h0, h1)):
                            pr = hi * 64
                            nc.tensor.matmul(
                                sp[:, hi * 512:(hi + 1) * 512],
                                kt_all[pr:pr + 64, j, kt * 128:(kt + 1) * 128],
                                qt_all[pr:pr + 64, j, qc * 512:(qc + 1) * 512],
                                start=True, stop=True,
                                tile_position=(pr, 0),
                            )
                        ex = exp_pool.tile([128, T_OWN], BF16, tag="exp")
                        nc.scalar.activation(out=ex, in_=sp, func=AF.Exp,
                                             scale=0.125)
                        if per_kt is not None and qc == 0:
                            per_kt(kt)
                        for hi, hh in enumerate((h0, h1)):
                            nc.tensor.matmul(
                                avs[hi],
                                VP[:, kt, hh * (HD + 1):(hh + 1) * (HD + 1)],
                                ex[:, hi * 512:(hi + 1) * 512],
                                start=(kt == 0), stop=(kt == T_ALL // 128 - 1),
                            )
                        if kt % 3 == 2 and fillers:
                            fillers.popleft()()
                    for hi, hh in enumerate((h0, h1)):
                        av = avs[hi]
                        ft, pr = hh // 2, (hh % 2) * 64
                        asl = aoT[pr:pr + 64, ft, qc * 512:(qc + 1) * 512]
                        nc.vector.tensor_copy(out=asl, in_=av[0:HD, :])
                        sums_sb = nrm_pool.tile([1, 512], FP32, tag="sums")
                        nc.vector.tensor_copy(out=sums_sb, in_=av[HD:HD + 1, :])
                        rec = nrm_pool.tile([1, 512], FP32, tag="rec")
                        nc.vector.reciprocal_approx_fast(out=rec, in_=sums_sb)
                        rec_bf = nrm_pool.tile([1, 512], BF16, tag="recbf")
                        nc.vector.tensor_copy(out=rec_bf, in_=rec)
                        drt = drpool.tile([1, 512], BF16, tag="drrec")
                        nc.sync.dma_start(out=drt, in_=rec_bf)
                        bcs = nrm_pool.tile([128, 512], BF16, tag="bcs")
                        nc.sync.dma_start(out=bcs,
                                          in_=drt.broadcast_to([128, 512]))
                        nc.vector.tensor_mul(
                            out=asl, in0=asl, in1=bcs[pr:pr + HD, :]
                        )

            vf1 = v_fillers(1)
            for f in kq_fillers(0):
                f()
            v0 = v_fillers(0)
            for j in range(8):
                fillers = deque()
                if j + 1 < 8:
                    fillers.extend(kq_fillers(j + 1))
                if j < 4:
                    fillers.extend(vf1[j * 4:(j + 1) * 4])
                if j == 0:
                    # pair 0 drives V(vc0, kt) just-in-time for its attnV
                    emit_pair_fill(0, fillers, per_kt=lambda kt: v0[kt]())
                else:
                    emit_pair_fill(j, fillers)
                # flush leftover fillers before next pair needs them
                while fillers:
                    fillers.popleft()()
        es_z1.close()  # z1T dead
        es_kqv.close()  # kt/qt/VP dead

        # ---- phase D: proj + residual -> x2 ----
        es_x2 = ExitStack()
        x2p = es_x2.enter_context(tc.tile_pool(name="x2p", bufs=1))
        x2_all = x2p.tile([128, T_OWN // 128, D], FP32, tag="x2")
        with tc.tile_pool(name="psD", bufs=2, space="PSUM") as ppsum:
            for tt in range(T_OWN // 128):
                xo = xpool.tile([128, D], FP32, tag="xa")
                nc.sync.dma_start(out=xo, in_=x[tt * 128:(tt + 1) * 128, :])
                for oc in range(2):
                    ps = ppsum.tile([128, 512], FP32, tag="pps")
                    for d in range(DT):
                        nc.tensor.matmul(
                            ps, aoT[:, d, tt * 128:(tt + 1) * 128],
                            projw_sb[:, d, oc * 512:(oc + 1) * 512],
                            start=(d == 0), stop=False,
                        )
                    nc.tensor.matmul(
                        ps, ones_bf, bproj_sb[:, oc * 512:(oc + 1) * 512],
                        start=False, stop=True,
                    )
                    nc.vector.scalar_tensor_tensor(
                        out=x2_all[:, tt, oc * 512:(oc + 1) * 512],
                        in0=ps, scalar=0.0, in1=xo[:, oc * 512:(oc + 1) * 512],
                        op0=ALU.bypass, op1=ALU.add,
                    )
        es_pjw.close()
        es_ao.close()  # aoT dead

        # ---- phase E: LN2 + transpose -> z2T ----
        es_z2 = ExitStack()
        z2p = es_z2.enter_context(tc.tile_pool(name="z2p", bufs=1))
        F8 = mybir.dt.float8e4
        z2a = z2p.tile([128, DT, T_OWN], F8, tag="z2a")
        z2b = z2p.tile([128, DT, T_OWN], F8, tag="z2b")
        with tc.tile_pool(name="psE", bufs=2, space="PSUM") as trpsE:
            for tt in range(T_OWN // 128):
                xt = x2_all[:, tt, :]
                stats = statpool.tile([128, 2, 6], FP32, tag="stats")
                nc.vector.bn_stats(out=stats[:, 0, :], in_=xt[:, 0:512])
                nc.vector.bn_stats(out=stats[:, 1, :], in_=xt[:, 512:1024])
                mv = statpool.tile([128, 2], FP32, tag="mv")
                nc.vector.bn_aggr(out=mv, in_=stats)
                sd = statpool.tile([128, 1], FP32, tag="sd")
                nc.scalar.activation(out=sd, in_=mv[:, 1:2], func=AF.Sqrt,
                                     bias=eps_sb)
                rinv = statpool.tile([128, 1], FP32, tag="rinv")
                nc.vector.reciprocal(out=rinv, in_=sd)
                zn = znpool.tile([128, D], BF16, tag="zn")
                nc.vector.tensor_scalar(
                    out=zn, in0=xt, scalar1=mv[:, 0:1], scalar2=rinv,
                    op0=ALU.subtract, op1=ALU.mult)
                for g in range(2):
                    ps = trpsE.tile([128, 4, 128], BF16, tag="trps")
                    for i in range(4):
                        nc.tensor.transpose(
                            ps[:, i, :],
                            zn[:, (4 * g + i) * 128:(4 * g + i + 1) * 128],
                            ident)
                    sl = (slice(None), slice(4 * g, 4 * g + 4),
                          slice(tt * 128, (tt + 1) * 128))
                    nc.vector.tensor_copy(out=z2a[sl], in_=ps)
                    nc.vector.tensor_tensor(
                        out=z2b[sl], in0=ps, in1=z2a[sl], op=ALU.subtract)

        # ---- phase F: MLP ----
        with (
            tc.tile_pool(name="yp", bufs=3) as ypool,
            tc.tile_pool(name="w1p", bufs=3) as w1_pool,
            tc.tile_pool(name="w2p", bufs=1) as w2_pool,
            tc.tile_pool(name="hp", bufs=1) as hpool,
            tc.tile_pool(name="psF", bufs=6, space="PSUM") as fpsum,
        ):
            w2_sb = w2_pool.tile([128, HID // 128, D], BF16, tag="w2sb")
            nc.sync.dma_start(out=w2_sb, in_=w2_t)
            for tc2 in range(2):
                hT = hpool.tile([128, HID // 128, 512], BF16, tag="hT")
                for jt in range(HID // 128):
                    w1fa = w1_pool.tile([128, DT, 128], F8, tag="w1fa")
                    nc.sync.dma_start(out=w1fa, in_=w1a.ap()[:, jt, :, :])
                    w1fb = w1_pool.tile([128, DT, 128], F8, tag="w1fb")
                    nc.sync.dma_start(out=w1fb, in_=w1b.ap()[:, jt, :, :])
                    ps = fpsum.tile([128, 512], FP32, tag="fps")
                    ts2 = slice(tc2 * 512, (tc2 + 1) * 512)
                    i = 0
                    for wf, zt in ((w1fa, z2a), (w1fb, z2a), (w1fa, z2b)):
                        for d in range(0, DT, 2):
                            nc.tensor.matmul(
                                ps, wf[:, d:d + 2, :], zt[:, d:d + 2, ts2],
                                start=(i == 0), stop=(i == 11),
                                perf_mode=mybir.MatmulPerfMode.DoubleRow)
                            i += 1
                    nc.scalar.activation(
                        out=hT[:, jt, :], in_=ps, func=AF.Gelu,
                        bias=b1_sb[:, jt:jt + 1], scale=1.0 / 32.0,
                    )
                for oc in range(2):
                    pss = []
                    for i in range(4):
                        ops_t = fpsum.tile([128, 512], FP32, tag="fps")
                        pss.append(ops_t)
                    for jt in range(HID // 128):
                        for tt in range(4):
                            nc.tensor.matmul(
                                pss[tt], hT[:, jt, tt * 128:(tt + 1) * 128],
                                w2_sb[:, jt, oc * 512:(oc + 1) * 512],
                                start=(jt == 0), stop=False,
                            )
                    for tt in range(4):
                        tglob = tc2 * 4 + tt
                        nc.tensor.matmul(
                            pss[tt], ones_bf, b2_sb[:, oc * 512:(oc + 1) * 512],
                            start=False, stop=True,
                        )
                        ys = ypool.tile([128, 512], FP32, tag="ys")
                        nc.vector.scalar_tensor_tensor(
                            out=ys, in0=pss[tt], scalar=0.0,
                            in1=x2_all[:, tglob, oc * 512:(oc + 1) * 512],
                            op0=ALU.bypass, op1=ALU.add,
                        )
                        nc.sync.dma_start(
                            out=y[tglob * 128:(tglob + 1) * 128,
                                  oc * 512:(oc + 1) * 512],
                            in_=ys,
                        )
        es_z2.close()
        es_x2.close()

    nc.compile()
    return nc


def prep_host_inputs(inputs):
    """Fold LN affine params into the adjacent matmul weights, cast to bf16,
    and build the 8 per-core input maps."""
    import ml_dtypes

    f32 = np.float32
    x = np.asarray(inputs["x"], f32)
    qkv_w = np.asarray(inputs["qkv_w"], f32)
    qkv_b = np.asarray(inputs["qkv_b"], f32)
    proj_w = np.asarray(inputs["proj_w"], f32)
    proj_b = np.asarray(inputs["proj_b"], f32)
    fc1_w = np.asarray(inputs["fc1_w"], f32)
    fc1_b = np.asarray(inputs["fc1_b"], f32)
    fc2_w = np.asarray(inputs["fc2_w"], f32)
    fc2_b = np.asarray(inputs["fc2_b"], f32)
    ln1_w = np.asarray(inputs["ln1_w"], f32)
    ln1_b = np.asarray(inputs["ln1_b"], f32)
    ln2_w = np.asarray(inputs["ln2_w"], f32)
    ln2_b = np.asarray(inputs["ln2_b"], f32)

    bf = ml_dtypes.bfloat16
    wqkv = (ln1_w[:, None] * qkv_w).astype(bf)
    bqkv = (qkv_b + ln1_b @ qkv_w).astype(f32)
    f8 = ml_dtypes.float8_e4m3
    w1s = (ln2_w[:, None] * fc1_w) * 32.0
    w1a_ = w1s.astype(f8)
    w1b_ = (w1s - w1a_.astype(f32)).astype(f8)
    w1l = lambda w: np.ascontiguousarray(
        w.reshape(8, 128, 32, 128).transpose(1, 2, 0, 3))
    b1 = (fc1_b + ln2_b @ fc1_w).astype(f32)

    shared = {
        "wqkv": wqkv, "bqkv": bqkv,
        "wproj": proj_w.astype(bf), "bproj": proj_b.astype(bf),
        "w1a": w1l(w1a_), "w1b": w1l(w1b_), "b1": b1,
        "w2": fc2_w.astype(bf), "b2": fc2_b.astype(bf),
    }
    in_maps = []
    for c in range(N_CORES):
        b, half = c // 2, c % 2
        own = x[b, half * 1024:(half + 1) * 1024]
        other = x[b, (1 - half) * 1024:(2 - half) * 1024]
        xc = np.concatenate([own, other], axis=0)
        in_maps.append({"x": np.ascontiguousarray(xc), **shared})
    return in_maps


# ---------------------------------------------------------------------------
# Cached PJRT runner (jit once, reuse across kernel() calls)
# ---------------------------------------------------------------------------
_CACHE = {}


def _get_runner():
    if "runner" in _CACHE:
        return _CACHE["runner"]

    from jax.experimental.shard_map import shard_map
    from jax.sharding import Mesh, PartitionSpec
    from concourse.bass2jax import (
        _bass_exec_p, install_neuronx_cc_hook, partition_id_tensor,
    )

    nc = build_nc()
    install_neuronx_cc_hook()

    partition_name = nc.partition_id_tensor.name if nc.partition_id_tensor else None
    in_names, out_names, out_avals, zero_shapes = [], [], [], []
    for alloc in nc.m.functions[0].allocations:
        if not isinstance(alloc, mybir.MemoryLocationSet):
            continue
        name = alloc.memorylocations[0].name
        if alloc.kind == "ExternalInput":
            if name != partition_name:
                in_names.append(name)
        elif alloc.kind == "ExternalOutput":
            shape = tuple(alloc.tensor_shape)
            dtype = mybir.dt.np(alloc.dtype)
            out_names.append(name)
            out_avals.append(jax.core.ShapedArray(shape, dtype))
            zero_shapes.append((shape, dtype))
    n_params = len(in_names)
    n_outs = len(out_names)
    all_in = list(in_names) + list(out_names)
    if partition_name is not None:
        all_in.append(partition_name)
    donate = tuple(range(n_params, n_params + n_outs))

    def _body(*args):
        operands = list(args)
        if partition_name is not None:
            operands.append(partition_id_tensor())
        outs = _bass_exec_p.bind(
            *operands,
            out_avals=tuple(out_avals),
            in_names=tuple(all_in),
            out_names=tuple(out_names),
            lowering_input_output_aliases=(),
            sim_require_finite=True,
            sim_require_nnan=True,
            nc=nc,
        )
        return tuple(outs)

    devices = jax.devices()[:N_CORES]
    mesh = Mesh(np.asarray(devices), ("core",))
    sharded = jax.jit(
        shard_map(
            _body, mesh=mesh,
            in_specs=(PartitionSpec("core"),) * (n_params + n_outs),
            out_specs=(PartitionSpec("core"),) * n_outs,
            check_rep=False,
        ),
        donate_argnums=donate, keep_unused=True,
    )

    def run(in_maps):
        concat_in = [
            np.concatenate([np.asarray(m[name]) for m in in_maps], axis=0)
            for name in in_names
        ]
        concat_zeros = [
            np.zeros((N_CORES * s[0], *s[1:]), dt) for (s, dt) in zero_shapes
        ]
        out_arrs = sharded(*concat_in, *concat_zeros)
        per_core = []
        for c in range(N_CORES):
            per_core.append({
                name: np.asarray(out_arrs[i]).reshape(
                    N_CORES, *out_avals[i].shape)[c]
                for i, name in enumerate(out_names)
            })
        return per_core

    _CACHE["runner"] = run
    return run


def kernel(**inputs) -> np.ndarray:
    run = _get_runner()
    in_maps = prep_host_inputs(inputs)
    results = run(in_maps)
    out = np.zeros((4, 2048, 1024), np.float32)
    for c in range(N_CORES):
        b, half = c // 2, c % 2
        out[b, half * 1024:(half + 1) * 1024] = results[c]["y"]
    return out



# revision 41
# speedup vs baseline: 1.0815x; 1.0674x over previous
"""Transformer block (pre-norm attention + MLP) on 8 TRN2 NeuronCores.

Sharding: 8 cores = 4 batch elements x 2 sequence halves (data parallel, no
collectives). Each core computes its 1024 "own" query tokens end-to-end and
redundantly builds K/V for the full 2048-token batch element. The k-token
order is permuted per core (own tokens first) so the SPMD program is
identical on every core — softmax over k is permutation invariant.

All matmuls run with bf16 operands (fp32 PSUM accumulation); the residual
path stays fp32. LayerNorm affine params are folded into the adjacent matmul
weights host-side. Softmax skips max-subtraction (|scores| <= ~10 here) and
gets its denominators for free from an appended ones-column on V.
"""

from collections import deque
from contextlib import ExitStack

import numpy as np

try:
    import jax
    jax.config.update("jax_compilation_cache_dir", "/tmp/jax_bass_cache")
    jax.config.update("jax_persistent_cache_min_compile_time_secs", 0.0)
    jax.config.update("jax_persistent_cache_min_entry_size_bytes", -1)
except Exception:
    import jax

import concourse.bacc as bacc
import concourse.bass as bass
import concourse.mybir as mybir
import concourse.tile as tile
from concourse.masks import make_identity

FP32 = mybir.dt.float32
BF16 = mybir.dt.bfloat16
AF = mybir.ActivationFunctionType
ALU = mybir.AluOpType

D = 1024          # model dim
DT = 8            # d tiles of 128
H = 16            # heads
HD = 64           # head dim
HID = 4096        # mlp hidden
T_ALL = 2048      # tokens per core incl. K/V-only tokens
T_OWN = 1024      # query/output tokens per core
EPS = 1e-6
N_CORES = 8


def _ln_transpose(nc, statp, znp, trp, src_getter, n_tiles, zt_out, eps_sb, ident,
                  copies_on_act=False, apply_on_gpsimd=False):
    """LayerNorm (w/b folded into the following matmul weights host-side)
    + PE transpose into zt_out [128, DT, n_tiles*128] bf16."""
    for tt in range(n_tiles):
        xt = src_getter(tt)  # [128, D] fp32 sbuf tile
        stats = statp.tile([128, 2, 6], FP32, tag="stats")
        nc.vector.bn_stats(out=stats[:, 0, :], in_=xt[:, 0:512])
        nc.vector.bn_stats(out=stats[:, 1, :], in_=xt[:, 512:1024])
        mv = statp.tile([128, 2], FP32, tag="mv")
        nc.vector.bn_aggr(out=mv, in_=stats)
        sd = statp.tile([128, 1], FP32, tag="sd")
        nc.scalar.activation(out=sd, in_=mv[:, 1:2], func=AF.Sqrt, bias=eps_sb)
        rinv = statp.tile([128, 1], FP32, tag="rinv")
        nc.vector.reciprocal(out=rinv, in_=sd)
        zn = znp.tile([128, D], BF16, tag="zn")
        eng = nc.gpsimd if apply_on_gpsimd else nc.vector
        eng.tensor_scalar(
            out=zn, in0=xt, scalar1=mv[:, 0:1], scalar2=rinv,
            op0=ALU.subtract, op1=ALU.mult,
        )
        if trp is None:
            # bf16 xbar DMA transpose: no PE work, no PSUM bounce
            for d in range(DT):
                nc.sync.dma_start_transpose(
                    out=zt_out[:, d, tt * 128:(tt + 1) * 128],
                    in_=zn[:, d * 128:(d + 1) * 128],
                )
        else:
            for g in range(2):
                ps = trp.tile([128, 4, 128], BF16, tag="trps")
                for i in range(4):
                    nc.tensor.transpose(
                        ps[:, i, :],
                        zn[:, (4 * g + i) * 128:(4 * g + i + 1) * 128], ident
                    )
                cp = nc.scalar.copy if copies_on_act else nc.vector.tensor_copy
                cp(out=zt_out[:, 4 * g:4 * g + 4, tt * 128:(tt + 1) * 128],
                   in_=ps)


def build_nc():
    nc = bacc.Bacc("TRN2", target_bir_lowering=False, debug=False, num_devices=N_CORES)

    x = nc.dram_tensor("x", [T_ALL, D], FP32, kind="ExternalInput")
    wqkv = nc.dram_tensor("wqkv", [D, 3 * D], BF16, kind="ExternalInput")
    bqkv = nc.dram_tensor("bqkv", [3 * D], FP32, kind="ExternalInput")
    wproj = nc.dram_tensor("wproj", [D, D], BF16, kind="ExternalInput")
    bproj = nc.dram_tensor("bproj", [D], BF16, kind="ExternalInput")
    w1a = nc.dram_tensor("w1a", [128, HID // 128, DT, 128], mybir.dt.float8e4, kind="ExternalInput")
    w1b = nc.dram_tensor("w1b", [128, HID // 128, DT, 128], mybir.dt.float8e4, kind="ExternalInput")
    b1 = nc.dram_tensor("b1", [HID], FP32, kind="ExternalInput")
    w2a = nc.dram_tensor("w2a", [128, HID // 128, D], mybir.dt.float8e4, kind="ExternalInput")
    w2b = nc.dram_tensor("w2b", [128, HID // 128, D], mybir.dt.float8e4, kind="ExternalInput")
    b2 = nc.dram_tensor("b2", [D], BF16, kind="ExternalInput")
    y = nc.dram_tensor("y", [T_OWN, D], FP32, kind="ExternalOutput")
    wqkv_t = wqkv.ap().rearrange("(dt p) f -> p dt f", p=128)   # [128, 8, 3072]

    with tile.TileContext(nc) as tc, ExitStack() as ctx:
        P = ctx.enter_context

        # ---- whole-kernel pools ----
        singles = P(tc.tile_pool(name="singles", bufs=1))
        xpool = P(tc.tile_pool(name="xin", bufs=3))
        statpool = P(tc.tile_pool(name="stat", bufs=6))
        znpool = P(tc.tile_pool(name="zn", bufs=4))
        es_ao = ExitStack()
        aop = es_ao.enter_context(tc.tile_pool(name="aop", bufs=1, side="right"))
        es_pjw = ExitStack()
        pjw_pool = es_pjw.enter_context(
            tc.tile_pool(name="pjw", bufs=1, side="right"))

        # ---- constants ----
        ident = singles.tile([128, 128], BF16)
        make_identity(nc, ident)
        ones_bf = singles.tile([1, 128], BF16)
        nc.vector.memset(ones_bf, 1.0)
        eps_sb = singles.tile([128, 1], FP32)
        nc.vector.memset(eps_sb, EPS)
        bq_sb = singles.tile([128, 24], FP32)
        nc.sync.dma_start(out=bq_sb, in_=bqkv.ap().rearrange("(f p) -> p f", p=128))
        b1_sb = singles.tile([128, 32], FP32)
        nc.sync.dma_start(out=b1_sb, in_=b1.ap().rearrange("(f p) -> p f", p=128))
        bproj_sb = singles.tile([1, D], BF16)
        nc.sync.dma_start(out=bproj_sb, in_=bproj.ap().rearrange("(o f) -> o f", o=1))
        b2_sb = singles.tile([1, D], BF16)
        nc.sync.dma_start(out=b2_sb, in_=b2.ap().rearrange("(o f) -> o f", o=1))
        # V-bias broadcast to all partitions [128, 1024]
        vbias_sb = singles.tile([128, D], FP32)
        nc.sync.dma_start(
            out=vbias_sb,
            in_=bass.AP(tensor=bqkv, offset=2 * D, ap=[[0, 128], [1, D]]),
        )

        # ---- phase A: LN1 + transpose -> z1T ----
        es_z1 = ExitStack()
        z1p = es_z1.enter_context(tc.tile_pool(name="z1p", bufs=1, side="right"))
        z1T = z1p.tile([128, DT, T_ALL], BF16, tag="z1T")

        def load_x(tt):
            xt = xpool.tile([128, D], FP32, tag="xa")
            nc.sync.dma_start(out=xt, in_=x[tt * 128:(tt + 1) * 128, :])
            return xt

        with tc.tile_pool(name="psA", bufs=2, space="PSUM") as trpsA:
            _ln_transpose(nc, statpool, znpool, trpsA, load_x, T_ALL // 128,
                          z1T, eps_sb, ident, copies_on_act=True)

        # proj weights: prefetch now (used ~300us later in phase D)
        projw_sb = pjw_pool.tile([128, DT, D], BF16, tag="projw")
        nc.sync.dma_start(
            out=projw_sb, in_=wproj.ap().rearrange("(dt p) f -> p dt f", p=128)
        )

        # ---- fused QKV + attention ----
        es_kqv = ExitStack()
        kqvp = es_kqv.enter_context(tc.tile_pool(name="kqvp", bufs=1))
        kt_all = kqvp.tile([128, DT, T_ALL], BF16, tag="kt")
        qt_all = kqvp.tile([128, DT, T_OWN], BF16, tag="qt")
        VP = kqvp.tile([128, 16, 16 * (HD + 1)], BF16, tag="vp")
        vp_ones = VP.rearrange("p k (h e) -> p k h e", e=HD + 1)[:, :, :, HD:HD + 1]
        nc.vector.memset(vp_ones, 1.0)
        aoT = aop.tile([128, DT, T_OWN], BF16, tag="aoT")

        with (
            tc.tile_pool(name="wq", bufs=2, side="right") as wq_pool,
            tc.tile_pool(name="wv", bufs=1, side="right") as wv_pool,
            tc.tile_pool(name="psB", bufs=2, space="PSUM") as qkpsum,
            tc.tile_pool(name="exps", bufs=4) as exp_pool,
            tc.tile_pool(name="nrm", bufs=2) as nrm_pool,
            tc.tile_pool(name="psCs", bufs=2, space="PSUM") as spsum,
            tc.tile_pool(name="psCa", bufs=2, space="PSUM") as avpsum,
            tc.tile_pool(name="drp", bufs=3, space="DRAM") as drpool,
        ):
            def kq_fillers(j):
                out = []
                state = {}
                for f in (8 + j, j):
                    nch = 4 if f >= 8 else 2
                    for tcn in range(nch):
                        def grp(f=f, tcn=tcn):
                            if f not in state:
                                wq_f = wq_pool.tile([128, DT, 128], BF16,
                                                    tag="wqf")
                                nc.sync.dma_start(
                                    out=wq_f,
                                    in_=wqkv_t[:, :, f * 128:(f + 1) * 128])
                                state[f] = wq_f
                            wq_f = state[f]
                            ps = qkpsum.tile([128, 512], FP32, tag="qkps")
                            for d in range(DT):
                                nc.tensor.matmul(
                                    ps, wq_f[:, d, :],
                                    z1T[:, d, tcn * 512:(tcn + 1) * 512],
                                    start=(d == 0), stop=(d == DT - 1),
                                )
                            if f >= 8:
                                dst = kt_all[:, f - 8, tcn * 512:(tcn + 1) * 512]
                            else:
                                dst = qt_all[:, f, tcn * 512:(tcn + 1) * 512]
                            nc.vector.tensor_scalar(
                                out=dst, in0=ps, scalar1=bq_sb[:, f:f + 1],
                                scalar2=None, op0=ALU.add,
                            )
                        out.append(grp)
                return out

            def v_fillers(vc):
                out = []
                state = {}
                for tt in range(T_ALL // 128):
                    def grp(tt=tt):
                        if "wv" not in state:
                            wv = wv_pool.tile([128, DT, 512], BF16, tag="wvf")
                            nc.sync.dma_start(
                                out=wv,
                                in_=wqkv_t[:, :, 2 * D + vc * 512:
                                           2 * D + (vc + 1) * 512])
                            state["wv"] = wv
                        wv = state["wv"]
                        ps = qkpsum.tile([128, 512], FP32, tag="qkps")
                        for d in range(DT):
                            nc.tensor.matmul(
                                ps, z1T[:, d, tt * 128:(tt + 1) * 128],
                                wv[:, d, :],
                                start=(d == 0), stop=(d == DT - 1),
                            )
                        dst = VP[:, tt, vc * 8 * (HD + 1):
                                 (vc + 1) * 8 * (HD + 1)]
                        dst = dst.rearrange(
                            "p (h e) -> p h e", e=HD + 1)[:, :, 0:HD]
                        srcp = ps.rearrange("p (h e) -> p h e", e=HD)
                        vb = vbias_sb[:, vc * 512:(vc + 1) * 512].rearrange(
                            "p (h e) -> p h e", e=HD)
                        nc.vector.scalar_tensor_tensor(
                            out=dst, in0=srcp, scalar=0.0, in1=vb,
                            op0=ALU.bypass, op1=ALU.add,
                        )
                    out.append(grp)
                return out

            def emit_pair_fill(j, fillers, per_kt=None):
                """Process head pair (2j, 2j+1) with scores row-packed into
                the two 64-row halves of the PE array (tile_position), one
                q-chunk at a time so PSUM stays within 8 banks. Filler
                psum-groups are popped every 3rd kt step."""
                h0, h1 = 2 * j, 2 * j + 1
                for qc in range(2):
                    avs = []
                    for hh in (h0, h1):
                        av_t = avpsum.tile([HD + 1, 512], FP32, tag="av")
                        avs.append(av_t)
                    for kt in range(T_ALL // 128):
                        sp = spsum.tile([128, T_OWN], FP32, tag="sps")
                        for hi, hh in enumerate((h0, h1)):
                            pr = hi * 64
                            nc.tensor.matmul(
                                sp[:, hi * 512:(hi + 1) * 512],
                                kt_all[pr:pr + 64, j, kt * 128:(kt + 1) * 128],
                                qt_all[pr:pr + 64, j, qc * 512:(qc + 1) * 512],
                                start=True, stop=True,
                                tile_position=(pr, 0),
                            )
                        ex = exp_pool.tile([128, T_OWN], BF16, tag="exp")
                        nc.scalar.activation(out=ex, in_=sp, func=AF.Exp,
                                             scale=0.125)
                        if per_kt is not None and qc == 0:
                            per_kt(kt)
                        for hi, hh in enumerate((h0, h1)):
                            nc.tensor.matmul(
                                avs[hi],
                                VP[:, kt, hh * (HD + 1):(hh + 1) * (HD + 1)],
                                ex[:, hi * 512:(hi + 1) * 512],
                                start=(kt == 0), stop=(kt == T_ALL // 128 - 1),
                            )
                        if kt % 3 == 2 and fillers:
                            fillers.popleft()()
                    for hi, hh in enumerate((h0, h1)):
                        av = avs[hi]
                        ft, pr = hh // 2, (hh % 2) * 64
                        asl = aoT[pr:pr + 64, ft, qc * 512:(qc + 1) * 512]
                        nc.vector.tensor_copy(out=asl, in_=av[0:HD, :])
                        sums_sb = nrm_pool.tile([1, 512], FP32, tag="sums")
                        nc.vector.tensor_copy(out=sums_sb, in_=av[HD:HD + 1, :])
                        rec = nrm_pool.tile([1, 512], FP32, tag="rec")
                        nc.vector.reciprocal_approx_fast(out=rec, in_=sums_sb)
                        rec_bf = nrm_pool.tile([1, 512], BF16, tag="recbf")
                        nc.vector.tensor_copy(out=rec_bf, in_=rec)
                        drt = drpool.tile([1, 512], BF16, tag="drrec")
                        nc.sync.dma_start(out=drt, in_=rec_bf)
                        bcs = nrm_pool.tile([128, 512], BF16, tag="bcs")
                        nc.sync.dma_start(out=bcs,
                                          in_=drt.broadcast_to([128, 512]))
                        nc.vector.tensor_mul(
                            out=asl, in0=asl, in1=bcs[pr:pr + HD, :]
                        )

            vf1 = v_fillers(1)
            for f in kq_fillers(0):
                f()
            v0 = v_fillers(0)
            for j in range(8):
                fillers = deque()
                if j + 1 < 8:
                    fillers.extend(kq_fillers(j + 1))
                if j < 4:
                    fillers.extend(vf1[j * 4:(j + 1) * 4])
                if j == 0:
                    # pair 0 drives V(vc0, kt) just-in-time for its attnV
                    emit_pair_fill(0, fillers, per_kt=lambda kt: v0[kt]())
                else:
                    emit_pair_fill(j, fillers)
                # flush leftover fillers before next pair needs them
                while fillers:
                    fillers.popleft()()
        es_z1.close()  # z1T dead
        es_kqv.close()  # kt/qt/VP dead

        # ---- phase D: proj + residual -> x2 ----
        es_x2 = ExitStack()
        x2p = es_x2.enter_context(tc.tile_pool(name="x2p", bufs=1))
        x2_all = x2p.tile([128, T_OWN // 128, D], FP32, tag="x2")
        with tc.tile_pool(name="psD", bufs=2, space="PSUM") as ppsum:
            for tt in range(T_OWN // 128):
                xo = xpool.tile([128, D], FP32, tag="xa")
                nc.sync.dma_start(out=xo, in_=x[tt * 128:(tt + 1) * 128, :])
                for oc in range(2):
                    ps = ppsum.tile([128, 512], FP32, tag="pps")
                    for d in range(DT):
                        nc.tensor.matmul(
                            ps, aoT[:, d, tt * 128:(tt + 1) * 128],
                            projw_sb[:, d, oc * 512:(oc + 1) * 512],
                            start=(d == 0), stop=False,
                        )
                    nc.tensor.matmul(
                        ps, ones_bf, bproj_sb[:, oc * 512:(oc + 1) * 512],
                        start=False, stop=True,
                    )
                    nc.vector.scalar_tensor_tensor(
                        out=x2_all[:, tt, oc * 512:(oc + 1) * 512],
                        in0=ps, scalar=0.0, in1=xo[:, oc * 512:(oc + 1) * 512],
                        op0=ALU.bypass, op1=ALU.add,
                    )
        es_pjw.close()
        es_ao.close()  # aoT dead

        # ---- phase E: LN2 + transpose -> z2T ----
        es_z2 = ExitStack()
        z2p = es_z2.enter_context(tc.tile_pool(name="z2p", bufs=1))
        F8 = mybir.dt.float8e4
        z2a = z2p.tile([128, DT, T_OWN], F8, tag="z2a")
        z2b = z2p.tile([128, DT, T_OWN], F8, tag="z2b")
        with tc.tile_pool(name="psE", bufs=2, space="PSUM") as trpsE:
            for tt in range(T_OWN // 128):
                xt = x2_all[:, tt, :]
                stats = statpool.tile([128, 2, 6], FP32, tag="stats")
                nc.vector.bn_stats(out=stats[:, 0, :], in_=xt[:, 0:512])
                nc.vector.bn_stats(out=stats[:, 1, :], in_=xt[:, 512:1024])
                mv = statpool.tile([128, 2], FP32, tag="mv")
                nc.vector.bn_aggr(out=mv, in_=stats)
                sd = statpool.tile([128, 1], FP32, tag="sd")
                nc.scalar.activation(out=sd, in_=mv[:, 1:2], func=AF.Sqrt,
                                     bias=eps_sb)
                rinv = statpool.tile([128, 1], FP32, tag="rinv")
                nc.vector.reciprocal(out=rinv, in_=sd)
                zn = znpool.tile([128, D], BF16, tag="zn")
                nc.vector.tensor_scalar(
                    out=zn, in0=xt, scalar1=mv[:, 0:1], scalar2=rinv,
                    op0=ALU.subtract, op1=ALU.mult)
                for g in range(2):
                    ps = trpsE.tile([128, 4, 128], BF16, tag="trps")
                    for i in range(4):
                        nc.tensor.transpose(
                            ps[:, i, :],
                            zn[:, (4 * g + i) * 128:(4 * g + i + 1) * 128],
                            ident)
                    sl = (slice(None), slice(4 * g, 4 * g + 4),
                          slice(tt * 128, (tt + 1) * 128))
                    nc.vector.tensor_copy(out=z2a[sl], in_=ps)
                    nc.vector.tensor_tensor(
                        out=z2b[sl], in0=ps, in1=z2a[sl], op=ALU.subtract)

        # ---- phase F: MLP ----
        with (
            tc.tile_pool(name="yp", bufs=3) as ypool,
            tc.tile_pool(name="w1p", bufs=3) as w1_pool,
            tc.tile_pool(name="w2p", bufs=1) as w2_pool,
            tc.tile_pool(name="hp", bufs=1) as hpool,
            tc.tile_pool(name="psF", bufs=6, space="PSUM") as fpsum,
        ):
            w2a_sb = w2_pool.tile([128, HID // 128, D], F8, tag="w2asb")
            nc.sync.dma_start(out=w2a_sb, in_=w2a.ap())
            w2b_sb = w2_pool.tile([128, HID // 128, D], F8, tag="w2bsb")
            nc.sync.dma_start(out=w2b_sb, in_=w2b.ap())
            for tc2 in range(2):
                ha = hpool.tile([128, HID // 128, 512], F8, tag="ha")
                hb = hpool.tile([128, HID // 128, 512], F8, tag="hb")
                for jt in range(HID // 128):
                    w1fa = w1_pool.tile([128, DT, 128], F8, tag="w1fa")
                    nc.sync.dma_start(out=w1fa, in_=w1a.ap()[:, jt, :, :])
                    w1fb = w1_pool.tile([128, DT, 128], F8, tag="w1fb")
                    nc.sync.dma_start(out=w1fb, in_=w1b.ap()[:, jt, :, :])
                    ps = fpsum.tile([128, 512], FP32, tag="fps")
                    ts2 = slice(tc2 * 512, (tc2 + 1) * 512)
                    i = 0
                    for wf, zt in ((w1fa, z2a), (w1fb, z2a), (w1fa, z2b)):
                        for d in range(0, DT, 2):
                            nc.tensor.matmul(
                                ps, wf[:, d:d + 2, :], zt[:, d:d + 2, ts2],
                                start=(i == 0), stop=(i == 11),
                                perf_mode=mybir.MatmulPerfMode.DoubleRow)
                            i += 1
                    hg = hpool.tile([128, 512], BF16, tag="hg", bufs=3)
                    nc.scalar.activation(
                        out=hg, in_=ps, func=AF.Gelu,
                        bias=b1_sb[:, jt:jt + 1], scale=1.0 / 32.0,
                    )
                    nc.vector.tensor_copy(out=ha[:, jt, :], in_=hg)
                    nc.vector.tensor_tensor(
                        out=hb[:, jt, :], in0=hg, in1=ha[:, jt, :],
                        op=ALU.subtract)
                for oc in range(2):
                    pss = []
                    for i in range(4):
                        ops_t = fpsum.tile([128, 512], FP32, tag="fps")
                        pss.append(ops_t)
                    osl = slice(oc * 512, (oc + 1) * 512)
                    for jp in range(HID // 256):
                        j0 = 2 * jp
                        for tt in range(4):
                            tsl = slice(tt * 128, (tt + 1) * 128)
                            for hh, ww in ((ha, w2a_sb), (ha, w2b_sb),
                                           (hb, w2a_sb)):
                                nc.tensor.matmul(
                                    pss[tt], hh[:, j0:j0 + 2, tsl],
                                    ww[:, j0:j0 + 2, osl],
                                    start=(jp == 0 and hh is ha
                                           and ww is w2a_sb),
                                    stop=False,
                                    perf_mode=mybir.MatmulPerfMode.DoubleRow,
                                )
                    for tt in range(4):
                        tglob = tc2 * 4 + tt
                        nc.tensor.matmul(
                            pss[tt], ones_bf, b2_sb[:, oc * 512:(oc + 1) * 512],
                            start=False, stop=True,
                        )
                        ys = ypool.tile([128, 512], FP32, tag="ys")
                        nc.vector.scalar_tensor_tensor(
                            out=ys, in0=pss[tt], scalar=1.0 / 64.0,
                            in1=x2_all[:, tglob, oc * 512:(oc + 1) * 512],
                            op0=ALU.mult, op1=ALU.add,
                        )
                        nc.sync.dma_start(
                            out=y[tglob * 128:(tglob + 1) * 128,
                                  oc * 512:(oc + 1) * 512],
                            in_=ys,
                        )
        es_z2.close()
        es_x2.close()

    nc.compile()
    return nc


def prep_host_inputs(inputs):
    """Fold LN affine params into the adjacent matmul weights, cast to bf16,
    and build the 8 per-core input maps."""
    import ml_dtypes

    f32 = np.float32
    x = np.asarray(inputs["x"], f32)
    qkv_w = np.asarray(inputs["qkv_w"], f32)
    qkv_b = np.asarray(inputs["qkv_b"], f32)
    proj_w = np.asarray(inputs["proj_w"], f32)
    proj_b = np.asarray(inputs["proj_b"], f32)
    fc1_w = np.asarray(inputs["fc1_w"], f32)
    fc1_b = np.asarray(inputs["fc1_b"], f32)
    fc2_w = np.asarray(inputs["fc2_w"], f32)
    fc2_b = np.asarray(inputs["fc2_b"], f32)
    ln1_w = np.asarray(inputs["ln1_w"], f32)
    ln1_b = np.asarray(inputs["ln1_b"], f32)
    ln2_w = np.asarray(inputs["ln2_w"], f32)
    ln2_b = np.asarray(inputs["ln2_b"], f32)

    bf = ml_dtypes.bfloat16
    wqkv = (ln1_w[:, None] * qkv_w).astype(bf)
    bqkv = (qkv_b + ln1_b @ qkv_w).astype(f32)
    f8 = ml_dtypes.float8_e4m3
    w1s = (ln2_w[:, None] * fc1_w) * 32.0
    w1a_ = w1s.astype(f8)
    w1b_ = (w1s - w1a_.astype(f32)).astype(f8)
    w1l = lambda w: np.ascontiguousarray(
        w.reshape(8, 128, 32, 128).transpose(1, 2, 0, 3))
    b1 = (fc1_b + ln2_b @ fc1_w).astype(f32)
    w2s = fc2_w * 64.0
    w2a_ = w2s.astype(f8)
    w2b_ = (w2s - w2a_.astype(f32)).astype(f8)
    w2l = lambda w: np.ascontiguousarray(
        w.reshape(32, 128, 1024).transpose(1, 0, 2))

    shared = {
        "wqkv": wqkv, "bqkv": bqkv,
        "wproj": proj_w.astype(bf), "bproj": proj_b.astype(bf),
        "w1a": w1l(w1a_), "w1b": w1l(w1b_), "b1": b1,
        "w2a": w2l(w2a_), "w2b": w2l(w2b_),
        "b2": (fc2_b * 64.0).astype(bf),
    }
    in_maps = []
    for c in range(N_CORES):
        b, half = c // 2, c % 2
        own = x[b, half * 1024:(half + 1) * 1024]
        other = x[b, (1 - half) * 1024:(2 - half) * 1024]
        xc = np.concatenate([own, other], axis=0)
        in_maps.append({"x": np.ascontiguousarray(xc), **shared})
    return in_maps


# ---------------------------------------------------------------------------
# Cached PJRT runner (jit once, reuse across kernel() calls)
# ---------------------------------------------------------------------------
_CACHE = {}


def _get_runner():
    if "runner" in _CACHE:
        return _CACHE["runner"]

    from jax.experimental.shard_map import shard_map
    from jax.sharding import Mesh, PartitionSpec
    from concourse.bass2jax import (
        _bass_exec_p, install_neuronx_cc_hook, partition_id_tensor,
    )

    nc = build_nc()
    install_neuronx_cc_hook()

    partition_name = nc.partition_id_tensor.name if nc.partition_id_tensor else None
    in_names, out_names, out_avals, zero_shapes = [], [], [], []
    for alloc in nc.m.functions[0].allocations:
        if not isinstance(alloc, mybir.MemoryLocationSet):
            continue
        name = alloc.memorylocations[0].name
        if alloc.kind == "ExternalInput":
            if name != partition_name:
                in_names.append(name)
        elif alloc.kind == "ExternalOutput":
            shape = tuple(alloc.tensor_shape)
            dtype = mybir.dt.np(alloc.dtype)
            out_names.append(name)
            out_avals.append(jax.core.ShapedArray(shape, dtype))
            zero_shapes.append((shape, dtype))
    n_params = len(in_names)
    n_outs = len(out_names)
    all_in = list(in_names) + list(out_names)
    if partition_name is not None:
        all_in.append(partition_name)
    donate = tuple(range(n_params, n_params + n_outs))

    def _body(*args):
        operands = list(args)
        if partition_name is not None:
            operands.append(partition_id_tensor())
        outs = _bass_exec_p.bind(
            *operands,
            out_avals=tuple(out_avals),
            in_names=tuple(all_in),
            out_names=tuple(out_names),
            lowering_input_output_aliases=(),
            sim_require_finite=True,
            sim_require_nnan=True,
            nc=nc,
        )
        return tuple(outs)

    devices = jax.devices()[:N_CORES]
    mesh = Mesh(np.asarray(devices), ("core",))
    sharded = jax.jit(
        shard_map(
            _body, mesh=mesh,
            in_specs=(PartitionSpec("core"),) * (n_params + n_outs),
            out_specs=(PartitionSpec("core"),) * n_outs,
            check_rep=False,
        ),
        donate_argnums=donate, keep_unused=True,
    )

    def run(in_maps):
        concat_in = [
            np.concatenate([np.asarray(m[name]) for m in in_maps], axis=0)
            for name in in_names
        ]
        concat_zeros = [
            np.zeros((N_CORES * s[0], *s[1:]), dt) for (s, dt) in zero_shapes
        ]
        out_arrs = sharded(*concat_in, *concat_zeros)
        per_core = []
        for c in range(N_CORES):
            per_core.append({
                name: np.asarray(out_arrs[i]).reshape(
                    N_CORES, *out_avals[i].shape)[c]
                for i, name in enumerate(out_names)
            })
        return per_core

    _CACHE["runner"] = run
    return run


def kernel(**inputs) -> np.ndarray:
    run = _get_runner()
    in_maps = prep_host_inputs(inputs)
    results = run(in_maps)
    out = np.zeros((4, 2048, 1024), np.float32)
    for c in range(N_CORES):
        b, half = c // 2, c % 2
        out[b, half * 1024:(half + 1) * 1024] = results[c]["y"]
    return out

